# revision 38
# baseline (speedup 1.0000x reference)
"""Trainium2 Bass kernel for the BDH-style recurrent block.

Strategy: data-parallel over B (8 batches -> 8 NeuronCores, no collectives).
The T=128-step scan is de-sequentialized into dense matmuls per core:

  u_t = relu(emb_t @ Dx.T)                                  (T,N)
  x_t = (XD*x_{t-1} + u_t)/s_t  with s_t = XD + sum(u_t)    (L1 norm; x>=0)
      => x = C @ u, C[t,s] = (1/s_s) exp(A_t - A_s), A_t = cumsum log(XD/s_r)
  a*_t = rho_{t-1} @ x_t = ((DecayMask . X X^T) @ ln(emb))_t   (rho_0 = 0)
  y_t  = relu(ln(a*_t) @ Dy.T) * x_t                        (x_t >= 0)
  v*_t = ln(y_t @ E.T)

All matmul operands are bf16 (f32 PSUM accumulation): halves the weight DMA
(HBM streaming is the dominant cost) and streams 1 col/cycle at any free
dim. The dataflow is transpose-free: X^T and Ycore^T are produced directly
in n-major layout by swapping the stationary operand. a* accumulates per
512-column Gram super-chunk, removing the full-Gram barrier.

The C matrix is built WITHOUT the -Q_t column broadcast: C'[s,t] =
exp(trik[s,t] + (Q_s - q_s)) is one ACT op (per-partition bias). The
dropped diag(e^{-Q_t}) row factor is absorbed exactly: e^{-Q_s} folds into
vn's LN output scale, and the LN eps is replaced per-row by eps*e^{2Q_t}
(LN of c*v with eps*c^2 equals LN of v with eps — exact identity), so the
a*/vraw row scales cancel inside the downstream LayerNorms.

Every ACT function is chosen from the one 'natural_log_exp_and_others'
table (rsqrt = exp(-.5 ln)), so the 1.3us table reload is paid once at
boot and never again.
"""

import math
from contextlib import ExitStack

import numpy as np
import ml_dtypes

BF16 = ml_dtypes.bfloat16

N = 2048
D = 256
B = 8
T = 128
XD = 0.97
UD = 0.97
LN_EPS = 1e-5
L1_EPS = 1e-12

# log-domain recentring: E[sum relu(N(0,1)) over 2048] + XD ~ 818.9
LNC2INV = 6.7065
C2 = math.exp(-LNC2INV)
K1 = LNC2INV - math.log(XD)
TRIK_FLOOR = -60.0   # exp(-60+colsc) ~ 1e-24 << any real C entry; in-table

KD = D // 128   # 2
KN = N // 128   # 16
NJ = N // 512   # 4

NCF = T + 1       # f32 consts: trik | xdvec
NCBIG = 6 * T     # bf16 block: embT(2T) | emb(D) | ustrict(T) | dmaskT(T)

_cache = {}


def _consts_f():
    r = np.arange(T)
    tri = r[None, :] - r[:, None]                                   # [s,t] t-s
    trik = np.where(tri >= 0,
                    np.maximum(-K1 * tri - LNC2INV, TRIK_FLOOR),
                    TRIK_FLOOR).astype(np.float32)
    xdvec = np.full((T, 1), C2 * XD, dtype=np.float32)
    xdvec[0, 0] = 0.0                                               # x_{-1} = 0
    return np.ascontiguousarray(np.concatenate([trik, xdvec], axis=1))


def _consts_b():
    r = np.arange(T)
    ustrict = (r[:, None] < r[None, :]).astype(BF16)                # [r,t] r<t
    pw = r[:, None] - 1 - r[None, :]                                # [t,s] t-1-s
    dmask = np.where(pw >= 0, UD ** np.maximum(pw, 0), 0.0)
    dmaskT = np.ascontiguousarray(dmask.T).astype(BF16)             # [s,t]
    return np.ascontiguousarray(np.concatenate([ustrict, dmaskT], axis=1))


def _pack_jk(wT):
    # (KD,128,N) k-major -> (128, [j(4), k(2), 512]) per-partition contiguous
    return np.ascontiguousarray(
        wT.reshape(KD, 128, NJ, 512).transpose(1, 2, 0, 3).reshape(128, KD * N))


def _split_multiwait(nc, mybir):
    """This walrus build caps sync waits per instruction (1 for regular
    instructions, 2 for EventSemaphore). Tile attaches more (e.g. the
    kernel-tail Drain waits on every live semaphore). Hoist excess waits
    onto same-engine NOPs placed immediately before the instruction —
    engine queues are sequential, so semantics are preserved."""
    n = 0
    for f in nc.m.functions:
        for bb in f.blocks:
            out = []
            changed = False
            for ins in bb.instructions:
                si = ins.sync_info
                ow = list(si.on_wait) if si is not None else []
                cap = 2 if ins.opcode == "EventSemaphore" else 1
                if len(ow) > cap:
                    sem_waits = [w for w in ow if w.sync_type == "semaphore"]
                    other = [w for w in ow if w.sync_type != "semaphore"]
                    keep = max(cap - len(other), 0)
                    hoist = sem_waits[:len(sem_waits) - keep] if keep else sem_waits
                    kept = sem_waits[len(hoist):] + other
                    assert len(kept) <= cap, (len(kept), cap, ins.opcode)
                    changed = True
                    for w in hoist:
                        n += 1
                        nop = mybir.InstNoOp(
                            name=f"wsplit-{n}",
                            sync_info=mybir.SyncInfo(on_wait=[w], on_update=[]),
                            bass_nofuse=True,
                            engine=ins.engine,
                        )
                        nc.register_instruction(nop, overwrite=True)
                        out.append(nop)
                    si.on_wait = kept
                out.append(ins)
            if changed:
                bb.instructions = out
    return nc


def _build():
    import concourse.bass as bass
    import concourse.mybir as mybir
    import concourse.tile as tile

    f32 = mybir.dt.float32
    bf16 = mybir.dt.bfloat16
    AF = mybir.ActivationFunctionType
    ALU = mybir.AluOpType
    AX = mybir.AxisListType

    from concourse.vector_clock import ScopedClock

    class _TrimTailTC(tile.TileContext):
        # Drop the second kernel-tail all-engine barrier: it only orders
        # the semaphore resets against engine halt, and nothing executes
        # after it. The first barrier (before resets) is kept, so resets
        # still happen on a quiesced machine and re-execution stays safe.
        def _drain_and_barrier(self, tick_clock, wait_clock):
            drain_inst = self.nc.sync.drain()
            wait_clock.add_sem_waits(
                drain_inst.ins, ScopedClock({None: tick_clock.global_clock})
            )
            self.nc.all_engine_barrier()
            assert self.sems is not None
            popped = self.nc._tile_sem_poison_stack.pop()
            assert popped is self._sem_poison
            self.nc.clear_and_free_semaphores(
                list(self.sems.allocated().values())
            )

    nc = bass.Bass()

    d_cbig = nc.dram_tensor("cbig", [128, NCBIG], bf16, kind="ExternalInput")
    d_cf = nc.dram_tensor("cf", [128, NCF], f32, kind="ExternalInput")
    d_dxT = nc.dram_tensor("dxT", [128, KD * N], bf16, kind="ExternalInput")
    d_dyT = nc.dram_tensor("dyT", [128, KD * N], bf16, kind="ExternalInput")
    d_eT = nc.dram_tensor("eT", [128, KN * D], bf16, kind="ExternalInput")
    d_out = nc.dram_tensor("out", [T, D], f32, kind="ExternalOutput")

    with _TrimTailTC(nc) as tc, ExitStack() as ctx:
        work = ctx.enter_context(tc.tile_pool(name="work", bufs=1))
        stats = ctx.enter_context(tc.tile_pool(name="stats", bufs=1))
        p_u = ctx.enter_context(tc.tile_pool(name="p_u", bufs=2, space="PSUM"))
        p_sq = ctx.enter_context(tc.tile_pool(name="p_sq", bufs=2, space="PSUM"))
        p_g = ctx.enter_context(tc.tile_pool(name="p_g", bufs=1, space="PSUM"))
        p_med = ctx.enter_context(tc.tile_pool(name="p_med", bufs=1,
                                               space="PSUM"))

        # ---- ACT table preload: one Ln at boot loads the
        # natural_log_exp_and_others set; every later ACT func is in-set.
        pre_sb = stats.tile([1, 1], f32)
        nc.gpsimd.memset(pre_sb[:], 1.0)
        pre_o = stats.tile([1, 1], f32)
        nc.scalar.activation(pre_o[:], pre_sb[:], AF.Ln)

        # const bias APs (walrus needs non-Copy act biases as APs)
        c_eps = stats.tile([T, 1], f32)
        nc.gpsimd.memset(c_eps[:], LN_EPS)
        c_lneps = stats.tile([T, 1], f32)
        nc.gpsimd.memset(c_lneps[:], math.log(LN_EPS))
        ones_col = stats.tile([128, 1], bf16)
        nc.gpsimd.memset(ones_col[:], 1.0)
        ones_1 = stats.tile([1, 1], f32)
        nc.gpsimd.memset(ones_1[:], 1.0)

        # ---- DMAs: qSP-HWDGE executes these in FIFO order, each striped
        # across the 16 SDMA engines at ~full HBM rate. Few, large pieces.
        cbig_sb = work.tile([128, NCBIG], bf16)
        nc.sync.dma_start(cbig_sb[:], d_cbig[:])
        embT_sb = cbig_sb[:, 0:2 * T]
        emb_sb = cbig_sb[:, 2 * T:2 * T + D]
        ustrict_sb = cbig_sb[:, 2 * T + D:3 * T + D]
        dmaskT_sb = cbig_sb[:, 3 * T + D:4 * T + D]

        # dxT streams immediately after the small const block (it gates u,
        # the head of the dependency chain); cf (trik/xdvec) is not needed
        # until the C' Exp, so it rides after.
        dxT_sb = work.tile([128, KD * N], bf16)
        for h in range(2):
            nc.sync.dma_start(dxT_sb[:, h * 2048:(h + 1) * 2048],
                              d_dxT[:, h * 2048:(h + 1) * 2048])
        cf_sb = work.tile([128, NCF], f32)
        nc.sync.dma_start(cf_sb[:], d_cf[:])
        trik_sb = cf_sb[:, 0:T]
        xdvec_sb = cf_sb[:, T:T + 1]

        dyT_sb = work.tile([128, KD * N], bf16)
        for h in range(2):
            nc.sync.dma_start(dyT_sb[:, h * 2048:(h + 1) * 2048],
                              d_dyT[:, h * 2048:(h + 1) * 2048])
        eT_sb = work.tile([128, KN * D], bf16)
        for h in range(2):
            nc.sync.dma_start(eT_sb[:, h * 2048:(h + 1) * 2048],
                              d_eT[:, h * 2048:(h + 1) * 2048])

        # ---- vn stats on GPSIMD (keeps ACT free for the u evacuations and
        # the q/ct chain; var = E[x^2] - m^2, benign for N(0,1) data).
        vjunk = work.tile([T, D], bf16)
        nc.gpsimd.tensor_tensor(vjunk[:], emb_sb, emb_sb, op=ALU.mult)
        msum = stats.tile([T, 1], f32)
        nc.vector.tensor_reduce(msum[:], emb_sb, axis=AX.X, op=ALU.add)
        ssq = stats.tile([T, 1], f32)
        nc.vector.tensor_reduce(ssq[:], vjunk[:], axis=AX.X, op=ALU.add)
        negm = stats.tile([T, 1], f32)
        nc.gpsimd.tensor_scalar_mul(negm[:], msum[:], -1.0 / D)
        nm2 = stats.tile([T, 1], f32)
        nc.gpsimd.tensor_tensor(nm2[:], negm[:], negm[:], op=ALU.mult)
        varv = stats.tile([T, 1], f32)
        nc.gpsimd.tensor_scalar(varv[:], ssq[:], 1.0 / D, nm2[:],
                                op0=ALU.mult, op1=ALU.subtract)
        lv_vn = stats.tile([T, 1], f32)
        nc.scalar.activation(lv_vn[:], varv[:], AF.Ln, bias=c_eps[:])
        rstd_vn = stats.tile([T, 1], f32)
        nc.scalar.activation(rstd_vn[:], lv_vn[:], AF.Exp, scale=-0.5)
        nmr_vn = stats.tile([T, 1], f32)
        nc.scalar.mul(nmr_vn[:], negm[:], rstd_vn[:])

        # ---- u = relu(emb @ Dx.T): t-major; each chunk's evac is split
        # DVE/ACT half-and-half so the tail chunk's latency halves.
        u_sb = work.tile([T, N], bf16)
        su_part = stats.tile([T, 2 * NJ], f32)

        def bridge(lhsT, rhs, cols):
            # Dep-pinned PE filler: reads real SBUF data so the scheduler
            # cannot hoist it ahead of the producer; output is never read.
            # Keeps the tensor engine dense across stalls so the DVFS clock
            # ramp is not reset (full clock needs ~4-5us of dense activity).
            ps = p_g.tile([T, T], f32, tag="g")
            nc.tensor.matmul(ps[:, 0:cols], lhsT, rhs, start=True, stop=True)

        for j in range(NJ):
            ps = p_u.tile([128, 512], f32, tag="pu")
            for k in range(KD):
                nc.tensor.matmul(
                    ps[:],
                    embT_sb[:, k * T:(k + 1) * T],
                    dxT_sb[:, j * 1024 + k * 512: j * 1024 + (k + 1) * 512],
                    start=(k == 0),
                    stop=(k == KD - 1),
                )
            cut = 384 if j == NJ - 1 else 256
            nc.vector.tensor_scalar(
                u_sb[:, j * 512:j * 512 + cut], ps[:, 0:cut], 0.0, 0.0,
                op0=ALU.max, op1=ALU.add, accum_out=su_part[:, 2 * j:2 * j + 1],
            )
            nc.scalar.activation(
                u_sb[:, j * 512 + cut:(j + 1) * 512], ps[:, cut:512], AF.Relu,
                accum_out=su_part[:, 2 * j + 1:2 * j + 2],
            )

        # ---- C' matrix: q -> strict cumsum -> one Exp with bias ----------
        su = stats.tile([T, 1], f32)
        nc.vector.tensor_reduce(su[:], su_part[:], axis=AX.X, op=ALU.add)
        q_sb = stats.tile([T, 1], bf16)
        nc.scalar.activation(q_sb[:], su[:], AF.Ln, scale=C2, bias=xdvec_sb[:])
        colsc = p_sq.tile([T, T], f32, tag="sq")            # Q_s - q_s column
        nc.tensor.matmul(colsc[:, 0:1], ustrict_sb[:], q_sb[:], start=True,
                         stop=True)
        colsc_sb = stats.tile([T, 1], f32)
        nc.vector.tensor_copy(colsc_sb[:], colsc[:, 0:1])
        ct_sb = work.tile([T, T], bf16)                     # C'[s,t]
        nc.scalar.activation(ct_sb[:], trik_sb[:], AF.Exp, bias=colsc_sb[:])

        # ---- scale-absorption factors (off critical path) ----------------
        qq = stats.tile([T, 1], f32)                        # Q_s inclusive
        nc.vector.tensor_tensor(qq[:], colsc[:, 0:1], q_sb[:], op=ALU.add)
        expQ = stats.tile([T, 1], f32)                      # e^{-Q_s}
        nc.scalar.activation(expQ[:], qq[:], AF.Exp, scale=-1.0)
        epsQ = stats.tile([T, 1], f32)                      # eps * e^{2 Q_t}
        nc.scalar.activation(epsQ[:], qq[:], AF.Exp, scale=2.0,
                             bias=c_lneps[:])
        rstd2 = stats.tile([T, 1], f32)
        nc.scalar.mul(rstd2[:], rstd_vn[:], expQ[:])
        nmr2 = stats.tile([T, 1], f32)
        nc.scalar.mul(nmr2[:], nmr_vn[:], expQ[:])
        vn_sb = work.tile([T, D], bf16)                     # e^{-Q} * LN(emb)
        nc.scalar.activation(vn_sb[:], emb_sb, AF.Identity,
                             scale=rstd2[:], bias=nmr2[:])

        # ---- X^T directly (n-major): XT_c = u_c^T(stationary) @ C'^T;
        # Gram super-chunks G_q = sum_c XT_c XT_c^T; a* += (G_q.dmask) @ vn.
        xt_sb = work.tile([128, N], bf16)
        apsT0 = p_med.tile([128, T], f32, tag="medT0")
        apsT1 = p_med.tile([128, T], f32, tag="medT1")
        apsT_ps = [apsT0, apsT1]
        gq_ps = {}

        def xt_block(q):
            for cc in range(4):
                c = 4 * q + cc
                # Alternate between p_sq and the now-dead p_u banks for a
                # 5-deep rotation (fewer WAR stalls on the evacuations).
                if c == 0:
                    tp = p_g.tile([T, T], f32, tag="g", name="tp_g")[:, 0:T]
                elif c % 2 == 0:
                    tp = p_sq.tile([128, T], f32, tag="sq")
                else:
                    tp = p_u.tile([128, 512], f32, tag="pu",
                                  name="tp_pu")[:, 0:T]
                nc.tensor.matmul(tp, u_sb[:, c * T:(c + 1) * T], ct_sb[:],
                                 start=True, stop=True)
                # GPSIMD cannot read PSUM; alternate DVE / ACT for evacs.
                # The last block goes all-ACT so the DVE reaches the Gram
                # mask ops promptly — their latency gates the a* matmuls,
                # and those stalls were resetting the PE clock ramp.
                if c % 2 == 0 and q < 3:
                    nc.vector.tensor_copy(xt_sb[:, c * T:(c + 1) * T], tp)
                else:
                    nc.scalar.copy(xt_sb[:, c * T:(c + 1) * T], tp)

        def g_block(q):
            g = p_g.tile([T, T], f32, tag="g")
            gq_ps[q] = g
            for cc in range(4):
                c = 4 * q + cc
                nc.tensor.matmul(g[:], xt_sb[:, c * T:(c + 1) * T],
                                 xt_sb[:, c * T:(c + 1) * T],
                                 start=(cc == 0), stop=(cc == 3))

        wt_tiles = {}

        def astar_mask(q):
            g = gq_ps.pop(q)
            wt = work.tile([T, T], bf16, tag=f"wt{q}")
            wt_tiles[q] = wt
            nc.vector.tensor_tensor(wt[:], g[:], dmaskT_sb[:], op=ALU.mult)

        def astar_mm(q):
            wt = wt_tiles[q]
            for k in range(KD):
                nc.tensor.matmul(apsT_ps[k][:],
                                 vn_sb[:, k * 128:(k + 1) * 128], wt[:],
                                 start=(q == 0), stop=(q == 3))

        # astar matmuls are deferred to the end: each a_q only needs wt_q
        # (DVE), so running a0..a2 while wt_3 lands keeps the PE gapless.
        xt_block(0)
        xt_block(1)
        g_block(0)
        xt_block(2)
        g_block(1)
        astar_mask(0)
        xt_block(3)
        g_block(2)
        astar_mask(1)
        g_block(3)
        astar_mask(2)
        astar_mask(3)
        for q in range(4):
            astar_mm(q)

        # ---- lna elimination: a* is an exact zero-mean row mix of LN
        # outputs, so LN(a*) = a* * rsqrt(var+eps). The rsqrt row scale is
        # folded into the vstar eps (c^2 rule), so a*^T streams RAW into
        # the Dy contraction with no LayerNorm on the critical path. The
        # variance comes later from a PE partition-reduction of apsT^2.
        apsT_sb = work.tile([128, KD * T], bf16)
        nc.vector.tensor_copy(apsT_sb[:, 0:T], apsT0[:])
        nc.scalar.copy(apsT_sb[:, T:2 * T], apsT1[:])
        sq_sb = work.tile([128, KD * T], bf16)
        nc.gpsimd.tensor_tensor(sq_sb[:], apsT_sb[:], apsT_sb[:], op=ALU.mult)

        # ---- Ycore^T directly (n-major): yc_c = Dy_c(stationary) @ a*T;
        # yT_c = relu(yc_c) * xT_c on DVE; vraw += yT_c^T @ eT_c. One chunk
        # of PE lookahead hides each chunk's DVE evac.
        yt_sb = work.tile([128, N], bf16)
        vps = p_med.tile([T, D], f32, tag="med")
        yc_ps = {}

        def yc_block(c):
            j = c // 4
            r = c % 4
            if c % 2 == 0:
                ps = p_sq.tile([128, T], f32, tag="sq")
            else:
                ps = p_u.tile([128, 512], f32, tag="pu",
                              name="yc_pu")[:, 0:T]
            yc_ps[c] = ps
            for k in range(KD):
                nc.tensor.matmul(
                    ps,
                    dyT_sb[:, j * 1024 + k * 512 + r * T:
                           j * 1024 + k * 512 + (r + 1) * T],
                    apsT_sb[:, k * T:(k + 1) * T],
                    start=(k == 0),
                    stop=(k == KD - 1),
                )

        def yt_evac(c):
            ps = yc_ps.pop(c)
            nc.vector.scalar_tensor_tensor(
                yt_sb[:, c * T:(c + 1) * T], ps, 0.0,
                xt_sb[:, c * T:(c + 1) * T], op0=ALU.max, op1=ALU.mult,
            )

        def vraw_block(c):
            nc.tensor.matmul(vps[:], yt_sb[:, c * T:(c + 1) * T],
                             eT_sb[:, c * D:(c + 1) * D],
                             start=(c == 0), stop=(c == KN - 1))

        yc_block(0)
        yt_evac(0)
        yc_block(1)
        yt_evac(1)
        for c in range(2, KN):
            yc_block(c)
            yt_evac(c)
            vraw_block(c - 2)
        vraw_block(KN - 2)
        vraw_block(KN - 1)

        # var_t = sum_d a*^2 / D via two tiny PE reductions (PE is idle
        # here); epsQ_vs = (var + epsQ) * epsQ feeds the vstar LN bias.
        vrow = p_u.tile([128, 512], f32, tag="pu", name="vrow")[0:1, 0:2 * T]
        nc.tensor.matmul(vrow, ones_col[:], sq_sb[:], start=True, stop=True)
        vrow_sb = stats.tile([1, 2 * T], f32)
        nc.vector.tensor_copy(vrow_sb[:], vrow)
        vcol = p_sq.tile([128, T], f32, tag="sq", name="vcol")[:, 0:1]
        nc.tensor.matmul(vcol, vrow_sb[0:1, 0:T], ones_1[:], start=True,
                         stop=False)
        nc.tensor.matmul(vcol, vrow_sb[0:1, T:2 * T], ones_1[:], start=False,
                         stop=True)
        veps_la = stats.tile([T, 1], f32)
        nc.vector.tensor_scalar(veps_la[:], vcol, 1.0 / D, epsQ[:],
                                op0=ALU.mult, op1=ALU.add)
        epsQ_vs = stats.tile([T, 1], f32)
        nc.vector.tensor_tensor(epsQ_vs[:], veps_la[:], epsQ[:], op=ALU.mult)

        # ---- v* = LN(vraw) with the same per-row eps, f32 out; the
        # normalize and writeback run split in halves so the first DMA
        # issues while the second half normalizes.
        vstar_sb = work.tile([T, D], f32)
        vs_s6 = stats.tile([T, 6], f32)
        nc.vector.bn_stats(vs_s6[:], vps[:])
        vs_mv = stats.tile([T, 2], f32)
        nc.vector.bn_aggr(vs_mv[:], vs_s6[:])
        vs_lv = stats.tile([T, 1], f32)
        nc.scalar.activation(vs_lv[:], vs_mv[:, 1:2], AF.Ln, bias=epsQ_vs[:])
        vs_rs = stats.tile([T, 1], f32)
        nc.scalar.activation(vs_rs[:], vs_lv[:], AF.Exp, scale=-0.5)
        for k in range(KD):
            nc.vector.tensor_scalar(
                vstar_sb[:, k * T:(k + 1) * T], vps[:, k * T:(k + 1) * T],
                vs_mv[:, 0:1], vs_rs[:], op0=ALU.subtract, op1=ALU.mult)
            nc.sync.dma_start(d_out[:, k * T:(k + 1) * T],
                              vstar_sb[:, k * T:(k + 1) * T])

    return _split_multiwait(nc, mybir)


def _numpy_fallback(embeddings, E, Dx, Dy, x_state, rho_state):
    # General-path reference (only used if initial states are nonzero).
    def ln(x):
        m = x.mean(-1, keepdims=True)
        v = ((x - m) ** 2).mean(-1, keepdims=True)
        return (x - m) / np.sqrt(v + LN_EPS)

    x_s = x_state.astype(np.float32).copy()
    rho = rho_state.astype(np.float32).copy()
    outs = np.zeros((B, T, D), dtype=np.float32)
    for t in range(T):
        v_prev = embeddings[:, t, :]
        x_upd = np.maximum(v_prev @ Dx.T, 0.0)
        x_t = XD * x_s + x_upd
        x_t = x_t / np.maximum(np.abs(x_t).sum(-1, keepdims=True), L1_EPS)
        a_star = np.einsum("bdn,bn->bd", rho, x_t)
        y_core = ln(a_star) @ Dy.T
        y_t = np.maximum(y_core, 0.0) * np.maximum(x_t, 0.0)
        outs[:, t, :] = ln(y_t @ E.T)
        vn = ln(v_prev)
        rho = UD * rho + np.einsum("bd,bn->bdn", vn, x_t)
        x_s = x_t
    return outs


def kernel(embeddings, E, Dx, Dy, x_state, rho_state):
    embeddings = np.ascontiguousarray(embeddings, dtype=np.float32)
    E = np.ascontiguousarray(E, dtype=np.float32)
    Dx = np.ascontiguousarray(Dx, dtype=np.float32)
    Dy = np.ascontiguousarray(Dy, dtype=np.float32)

    if np.any(x_state) or np.any(rho_state):
        return _numpy_fallback(embeddings, E, Dx, Dy,
                               np.asarray(x_state, np.float32),
                               np.asarray(rho_state, np.float32))

    from concourse.bass_utils import run_bass_kernel_spmd

    if "nc" not in _cache:
        _cache["nc"] = _build()
    nc = _cache["nc"]

    cf = _consts_f()
    cb = _consts_b()
    dxT = _pack_jk(Dx.T.reshape(KD, 128, N)).astype(BF16)
    dyT = _pack_jk(Dy.T.reshape(KD, 128, N)).astype(BF16)
    eT = np.ascontiguousarray(
        E.T.reshape(KN, 128, D).transpose(1, 0, 2).reshape(128, KN * D)
    ).astype(BF16)

    in_maps = []
    for b in range(B):
        emb_b = embeddings[b]
        embT_b = np.ascontiguousarray(
            emb_b.T.reshape(KD, 128, T).transpose(1, 0, 2).reshape(128, KD * T)
        ).astype(BF16)
        cbig = np.ascontiguousarray(np.concatenate(
            [embT_b, emb_b.astype(BF16), cb], axis=1))
        in_maps.append({
            "cbig": cbig,
            "cf": cf,
            "dxT": dxT,
            "dyT": dyT,
            "eT": eT,
        })

    res = run_bass_kernel_spmd(nc, in_maps, list(range(B)))
    _cache["last_results"] = res
    return np.stack([res.results[i]["out"] for i in range(B)])
